# revision 61
# baseline (speedup 1.0000x reference)
"""Single-head causal attention (B=4, T=4096, E=1024, H=128) on 8 TRN2 cores.

Sharding: 2 cores per batch, "folded triangle" split of the causal work.
Chunk0 = queries [0,2048), chunk1 = [2048,4096).
  core (b, 0): TRI : chunk0 q vs k in [0, q]        (causal triangle)
               RECT: chunk1 q vs k in [0, 1024)     (no mask)
  core (b, 1): TRI : chunk1 q vs k in [2048, q]     (causal triangle)
               RECT: chunk1 q vs k in [1024, 2048)  (no mask)
Both cores run the *identical* program; only the data differs. Outputs are
unnormalized accumulators acc = P@V and row-sums l = P@1; host sums
partials for chunk1 and normalizes.

Speed design (cost-model driven):
  - x/W arrive as fp8 e4m3; Q/K/V projections run as fp8 DoubleRow
    matmuls (0.5 cyc/row, K=256 per call) -> 4x faster than bf16.
  - V is projected directly in [token, head] orientation so no PE
    transposes are needed for the AV operand.
  - exp() writes P^T as fp8; AV and the row-sum ("ones") matmuls run as
    fp8 DoubleRow over k-tile pairs -> 4x faster than bf16 pairs.
  - S matmuls stay bf16 (qt/kt evacuated from the fp8 projections).
  - Causal masking: S and exp cover only columns q >= 128*dp of each
    diagonal tile; after exp, affine_select on Pool zeroes the invalid
    region of P^T directly, keeping the mask OFF the S->exp critical
    chain (a pre-exp mask add would serialize every diagonal exp).
  - Precision: short softmax rows amplify elementwise errors (row 0 is
    exactly v0), so the first triangle job (q in [0,512), k in [0,256))
    runs fully in bf16 from a bf16 "island" projection of x[0:512).
    All other rows average >=257 softmax terms where fp8 noise ~1e-2.
  - Projections are interleaved one-unit-per-job into the attention
    stream; tri and rect q-blocks alternate so PE load stays smooth and
    the Activation engine (exp, the bottleneck) never starves.
  - Startup: x block 0 is split across the ACT and SP DMA queues and
    block 1 rides Pool, in parallel with the weight loads on SP; PSUM
    evacuations run on DVE (GPSIMD cannot touch PSUM on real silicon).
    First exp fires ~5us in; the tail drain is special-cased so the
    final acc/l DMAs overlap the last accumulator evacuation.
"""

import sys

if "/opt/trn_rl_repo" not in sys.path:
    sys.path.insert(0, "/opt/trn_rl_repo")

import numpy as np
import ml_dtypes

import concourse.bacc as bacc
import concourse.bass as bass
import concourse.mybir as mybir
from concourse import masks, tile
from concourse.bass_utils import run_bass_kernel_spmd

E = 1024
H = 128
T = 4096
CH = 2048            # chunk length
TQ = 4096            # q tokens per core: [tri own-chunk 2048 | rect chunk1 2048]
RK = 1024            # rect-k region length
NKT_RK = RK // 128   # 8 k-tiles in the rect-k region
SCALE = 1.0 / np.sqrt(np.float32(H))

F32 = mybir.dt.float32
BF16 = mybir.dt.bfloat16
FP8 = mybir.dt.float8e4
NP8 = ml_dtypes.float8_e4m3
DR = mybir.MatmulPerfMode.DoubleRow

EC = E // 128       # 8 contraction chunks for the projections
NB_Q = TQ // 512    # 8 q blocks
NB_RK = RK // 512   # 2 rect-k blocks

_CACHED = {}
TRACE = False
TRACE_CORES = None
LAST_RESULTS = None


def _build(loop_n=None):
    nc = bacc.Bacc("TRN2", target_bir_lowering=False, debug=False, num_devices=8)
    # host-tiled fp8: [128p, tb, ec, 512]
    xq_in = nc.dram_tensor("xq_in", [128, NB_Q, EC, 512], FP8, kind="ExternalInput").ap()
    xrk_in = nc.dram_tensor("xrk_in", [128, NB_RK, EC, 512], FP8, kind="ExternalInput").ap()
    # bf16 island x: tri tokens [0,512)
    xb0w_in = nc.dram_tensor("xb0w_in", [128, EC, 512], BF16, kind="ExternalInput").ap()
    w8_in = {}
    wb_in = {}
    for name in ("q", "k", "v"):
        w8_in[name] = nc.dram_tensor(f"w8{name}_in", [128, EC, H], FP8, kind="ExternalInput").ap()
        wb_in[name] = nc.dram_tensor(f"wb{name}_in", [128, EC, H], BF16, kind="ExternalInput").ap()
    acc_out = nc.dram_tensor("acc_out", [H, TQ], F32, kind="ExternalOutput").ap()
    warm_out = nc.dram_tensor("warm_out", [1, 1], F32, kind="ExternalOutput").ap()
    l_out = nc.dram_tensor("l_out", [1, TQ], F32, kind="ExternalOutput").ap()

    import contextlib

    with tile.TileContext(nc) as tc:
        loop_cm = tc.For_i(0, loop_n, 1) if loop_n else contextlib.nullcontext()
        with (
            tc.tile_pool(name="const", bufs=1) as constp,
            tc.tile_pool(name="wpool", bufs=1) as wpool,
            tc.tile_pool(name="xin", bufs=6) as xin,
            tc.tile_pool(name="proj", bufs=1) as projp,
            tc.tile_pool(name="ppool", bufs=4) as ppool,
            tc.tile_pool(name="outp", bufs=4) as outp,
            tc.tile_pool(name="psS", bufs=2, space="PSUM") as psS,
            tc.tile_pool(name="psY", bufs=1, space="PSUM") as psY,
            tc.tile_pool(name="psP", bufs=2, space="PSUM") as psP,
            tc.tile_pool(name="psL", bufs=1, space="PSUM") as psL,
            loop_cm,
        ):
            # ---- parallel-queue startup: x blocks 0/1 ride ACT+SP / Pool --
            xb_tri = [None] * 4
            xb_rk = [None] * NB_RK
            xb_rq = {}

            def load_x(dst, idx, src, slot, eng):
                xb = xin.tile([128, EC, 512], FP8, tag="xb", name=f"xb{slot}")
                eng.dma_start(xb[:], src)
                dst[idx] = xb

            # Pool: constants first (warm needs ones_f), then x block 1
            ones_f = constp.tile([128, 2, 128], F32, tag="ones32")
            nc.gpsimd.memset(ones_f[:], 1.0)
            load_x(xb_tri, 1, xq_in[:, 1], "t1", nc.gpsimd)

            # x block 0 is split across the ACT and SP queues so the first
            # projection starts ~0.8us earlier; ACT then warms the Exp table
            xb0 = xin.tile([128, EC, 512], FP8, tag="xb", name="xbt0")
            nc.scalar.dma_start(xb0[:, : EC // 2], xq_in[:, 0, : EC // 2])
            xb_tri[0] = xb0
            warm = constp.tile([1, 1], F32, tag="warm")
            nc.scalar.activation(
                warm[:], ones_f[:1, 0, :1], mybir.ActivationFunctionType.Exp,
                scale=1.0
            )
            nc.gpsimd.dma_start(warm_out, warm[:])

            ones8 = constp.tile([128, 2, 128], FP8, tag="ones8")
            nc.vector.tensor_copy(ones8[:], ones_f[:])
            onesb = constp.tile([128, 1], BF16, tag="onesb")
            nc.vector.tensor_copy(onesb[:], ones_f[:, 0, :1])

            # ---- SP DMA queue (in-order), sequenced against unit deadlines
            w8 = {}
            wb = {}

            def load_w(name, fp8):
                d = w8 if fp8 else wb
                wt = wpool.tile([128, EC, H], FP8 if fp8 else BF16,
                                tag=f"w{'8' if fp8 else 'b'}{name}")
                nc.sync.dma_start(wt[:], (w8_in if fp8 else wb_in)[name])
                d[name] = wt

            xb0w = wpool.tile([128, EC, 512], BF16, tag="xb0w")

            load_w("k", True)
            nc.sync.dma_start(xb0[:, EC // 2 :], xq_in[:, 0, EC // 2 :])
            load_w("q", True)
            load_w("v", True)
            load_x(xb_tri, 2, xq_in[:, 2], "t2", nc.sync)
            load_x(xb_rk, 0, xrk_in[:, 0], "rk0", nc.sync)
            load_x(xb_rq, 4, xq_in[:, 4], "rq4", nc.sync)
            load_x(xb_rk, 1, xrk_in[:, 1], "rk1", nc.sync)
            load_x(xb_rq, 5, xq_in[:, 5], "rq5", nc.sync)
            load_x(xb_tri, 3, xq_in[:, 3], "t3", nc.sync)
            load_x(xb_rq, 6, xq_in[:, 6], "rq6", nc.sync)
            nc.sync.dma_start(xb0w[:], xb0w_in)
            load_w("q", False)
            load_w("k", False)
            load_w("v", False)
            load_x(xb_rq, 7, xq_in[:, 7], "rq7", nc.sync)

            # ---- projection targets ----
            qt = projp.tile([128, TQ], BF16, tag="qt")
            kt = projp.tile([128, RK + CH], BF16, tag="kt")
            vsb = projp.tile([128, NKT_RK + CH // 128, 128], FP8, tag="v")
            qti = projp.tile([128, 512], BF16, tag="qti")
            kti = projp.tile([128, 256], BF16, tag="kti")
            vbi = projp.tile([128, 2, 128], BF16, tag="vbi")

            def proj_qk(name, xb, dcol, pool=None):
                """fp8 DoubleRow: out[name][:, dcol:dcol+512] = W^T x  [h, tok]."""
                if pool is None:
                    ps = psP.tile([128, 512], F32, tag="psproj")
                else:  # upfront: borrow idle psS banks ([128,1024], use half)
                    ps = pool.tile([128, 1024], F32, tag="s", name="pswide")
                for i in range(EC // 2):
                    nc.tensor.matmul(
                        ps[:, :512], w8[name][:, 2 * i : 2 * i + 2, :],
                        xb[:, 2 * i : 2 * i + 2, :],
                        start=(i == 0), stop=(i == EC // 2 - 1), perf_mode=DR,
                    )
                dst = qt if name == "q" else kt
                nc.vector.tensor_copy(dst[:, dcol : dcol + 512], ps[:, :512])

            def proj_v(xb, kv0, pool=None):
                """fp8 DoubleRow, [tok, h] orientation: vsb tiles kv0/128..+4."""
                if pool is None:
                    ps = psP.tile([128, 4, 128], F32, tag="psproj", name="psv")
                    pv = ps
                else:
                    ps = pool.tile([128, 1024], F32, tag="s", name="pswide")
                    pv = None
                for t in range(4):
                    out = ps[:, 128 * t : 128 * (t + 1)] if pv is None else ps[:, t, :]
                    for i in range(EC // 2):
                        nc.tensor.matmul(
                            out,
                            xb[:, 2 * i : 2 * i + 2, 128 * t : 128 * (t + 1)],
                            w8["v"][:, 2 * i : 2 * i + 2, :],
                            start=(i == 0), stop=(i == EC // 2 - 1), perf_mode=DR,
                        )
                nc.vector.tensor_copy(
                    vsb[:, kv0 // 128 : kv0 // 128 + 4, :],
                    ps[:, :512] if pv is None else ps[:],
                )

            isl_ps = {}

            def isl_q(half, ecs):
                """Quarter-unit of the bf16 island q projection: one 256-col
                half, one 4-ec slice of the contraction. The psP tile is held
                open across the four consecutive units."""
                def f():
                    if half == 0 and ecs == 0:
                        isl_ps["q"] = psP.tile([128, 512], F32, tag="psproj",
                                               name="psislq")
                    ps = isl_ps["q"]
                    c0, c1 = 256 * half, 256 * half + 256
                    for ec in range(4 * ecs, 4 * ecs + 4):
                        nc.tensor.matmul(ps[:, c0:c1], wb["q"][:, ec, :],
                                         xb0w[:, ec, c0:c1],
                                         start=(ec == 0), stop=(ec == EC - 1))
                    if half == 1 and ecs == 1:
                        nc.vector.tensor_copy(qti[:], ps[:])
                return f

            def isl_k(ecs):
                def f():
                    if ecs == 0:
                        isl_ps["k"] = psP.tile([128, 512], F32, tag="psproj",
                                               name="psislk")
                    ps = isl_ps["k"]
                    for ec in range(4 * ecs, 4 * ecs + 4):
                        nc.tensor.matmul(ps[:, :256], wb["k"][:, ec, :],
                                         xb0w[:, ec, :256],
                                         start=(ec == 0), stop=(ec == EC - 1))
                    if ecs == 1:
                        nc.vector.tensor_copy(kti[:], ps[:, :256])
                return f

            def isl_v(t):
                def f():
                    if t == 0:
                        isl_ps["v"] = psP.tile([128, 4, 128], F32, tag="psproj",
                                               name="psv")
                    ps = isl_ps["v"]
                    for ec in range(EC):
                        nc.tensor.matmul(
                            ps[:, t, :], xb0w[:, ec, 128 * t : 128 * (t + 1)],
                            wb["v"][:, ec, :],
                            start=(ec == 0), stop=(ec == EC - 1))
                    if t == 1:
                        nc.vector.tensor_copy(vbi[:], ps[:, :2, :])
                return f

            # ---- upfront projections: the bare minimum for job 0's S.
            # These run on the still-idle psS banks so the single psP slot
            # doesn't serialize matmul->evac->matmul at startup.
            proj_qk("k", xb_tri[0], RK + 0, pool=psS)
            proj_qk("q", xb_tri[1], 512, pool=psS)

            # ---- jobs ----
            # job = (qb, k0, d0, k1, d1, first, last, island)
            def tri_jobs(qb):
                kts = []
                for j in range(4 * qb + 4):
                    dp = j - 4 * qb if j >= 4 * qb else None
                    kts.append((NKT_RK + j, dp))
                out = []
                for i in range(len(kts) // 2):
                    (k0, d0), (k1, d1) = kts[2 * i], kts[2 * i + 1]
                    out.append([qb, k0, d0, k1, d1, i == 0,
                                2 * i + 2 == len(kts), False])
                return out

            def rect_jobs(qb):
                return [[qb, 2 * i, None, 2 * i + 1, None,
                         i == 0, 2 * i + 2 == NKT_RK, False]
                        for i in range(NKT_RK // 2)]

            qb0 = tri_jobs(0)
            qb0[0][7] = True  # island job
            # interleave tri and rect q-blocks so the projection units
            # spread evenly across the whole ACT-paced job stream
            jobs = (tri_jobs(1) + tri_jobs(2) + rect_jobs(4) + rect_jobs(5)
                    + tri_jobs(3) + qb0 + rect_jobs(6) + rect_jobs(7))

            # one proj/load unit per job index (deadline-safe schedule);
            # lambdas late-bind the xb tiles loaded by the DMA sequence.
            pending = [
                lambda: proj_v(xb_tri[0], RK + 0),             # 0  AV(0)
                lambda: proj_qk("k", xb_tri[1], RK + 512),     # 1  S(2)
                lambda: proj_v(xb_tri[1], RK + 512),           # 2  AV(2)
                lambda: proj_qk("q", xb_tri[2], 1024),         # 3  S(4)
                lambda: proj_qk("k", xb_tri[2], RK + 1024),    # 4  S(8)
                lambda: proj_qk("q", xb_tri[0], 0),            # 5  frees t0 buf
                lambda: proj_v(xb_tri[2], RK + 1024),          # 6  AV(8)
                # one unit per slot; only the short (2,3)-diagonal slots
                # (jobs 9, 25, 27) stay unit-free — (0,1) pairs now run a
                # single full-width exp so their ACT budget is normal
                lambda: proj_qk("k", xb_rk[0], 0),             # 7  S(10)
                lambda: proj_qk("q", xb_rq[4], 2048),          # 8  S(10)
                None,                                          # 9
                lambda: proj_v(xb_rk[0], 0),                   # 10 AV(10)
                lambda: proj_qk("k", xb_rk[1], 512),           # 11 S(12)
                lambda: proj_v(xb_rk[1], 512),                 # 12 AV(12)
                lambda: proj_qk("q", xb_rq[5], 2560),          # 13 S(14)
                lambda: proj_qk("q", xb_tri[3], 1536),         # 14 S(18)
                lambda: proj_qk("k", xb_tri[3], RK + 1536),    # 15 S(24)
                lambda: proj_v(xb_tri[3], RK + 1536),          # 16 AV(24)
                isl_q(0, 0),                                   # 17
                isl_q(0, 1),                                   # 18
                isl_q(1, 0),                                   # 19
                isl_q(1, 1),                                   # 20
                isl_k(0),                                      # 21
                isl_k(1),                                      # 22
                isl_v(0),                                      # 23
                isl_v(1),                                      # 24 AV(26)
                None,                                          # 25
                lambda: proj_qk("q", xb_rq[6], 3072),          # 26 S(28)
                None,                                          # 27
                lambda: proj_qk("q", xb_rq[7], 3584),          # 28 S(32)
            ]
            units = {}
            for i, u in enumerate(pending):
                if u is not None:
                    units[i] = u if isinstance(u, list) else [u]

            n = len(jobs)
            ss_t = [None] * n
            pt_t = [None] * n
            ybank = {}

            def sel_mask(engine, pth, dp):
                # zero pt where q < k + 128*dp (invalid region incl. the
                # columns exp never wrote)
                engine.affine_select(
                    out=pth, in_=pth,
                    compare_op=mybir.AluOpType.is_ge,
                    fill=0.0, base=-128 * dp,
                    pattern=[[1, 512]], channel_multiplier=-1,
                )

            def s_stage(j):
                qb, k0, d0, k1, d1, _, _, isl = jobs[j]
                # (0,1) diagonal pairs: a single full-width exp beats two
                # shortened ones (one bubble instead of two, 896 vs 1024
                # cols) and releases the psS buffer in one instruction, so
                # S runs full-width for them; (2,3) pairs keep the split.
                single = d0 == 0
                c0 = 0 if single else 128 * (d0 or 0)
                c1 = 0 if single else 128 * (d1 or 0)
                if isl:
                    ktiles = (kti[:, :128], kti[:, 128:])
                    qs = qti[:]
                else:
                    ktiles = (kt[:, 128 * k0 : 128 * (k0 + 1)],
                              kt[:, 128 * k1 : 128 * (k1 + 1)])
                    qs = qt[:, 512 * qb : 512 * (qb + 1)]
                pt = ppool.tile([128, 2, 512], BF16 if isl else FP8,
                                tag="pt0" if isl else "pt")
                pt_t[j] = pt
                ss = psS.tile([128, 1024], F32, tag="s")
                nc.tensor.matmul(ss[:, c0:512], ktiles[0], qs[:, c0:],
                                 start=True, stop=True)
                nc.tensor.matmul(ss[:, 512 + c1 :], ktiles[1], qs[:, c1:],
                                 start=True, stop=True)
                if d0 is None or single:
                    nc.scalar.activation(
                        pt[:], ss[:], mybir.ActivationFunctionType.Exp,
                        scale=SCALE)
                else:
                    nc.scalar.activation(
                        pt[:, 0, c0:], ss[:, c0:512],
                        mybir.ActivationFunctionType.Exp, scale=SCALE)
                    nc.scalar.activation(
                        pt[:, 1, c1:], ss[:, 512 + c1 :],
                        mybir.ActivationFunctionType.Exp, scale=SCALE)
                if d0 is not None:
                    sel_mask(nc.gpsimd, pt[:, 0, :], d0)
                    sel_mask(nc.gpsimd, pt[:, 1, :], d1)

            def av_stage(j):
                qb, k0, d0, k1, d1, first, last, isl = jobs[j]
                if first:
                    ys = psY.tile([128, 512], F32, tag="y", name=f"ys{qb}")
                    ls = psL.tile([128, 512], F32, tag="ls", name=f"ls{qb}")
                    ybank[qb] = (ys, ls)
                ys, ls = ybank[qb]
                pt = pt_t[j]
                if isl:
                    nc.tensor.matmul(ys[:], vbi[:, 0, :], pt[:, 0, :],
                                     start=first, stop=False)
                    nc.tensor.matmul(ys[:], vbi[:, 1, :], pt[:, 1, :],
                                     start=False, stop=last)
                    nc.tensor.matmul(ls[:1, :], onesb[:], pt[:, 0, :],
                                     start=first, stop=False)
                    nc.tensor.matmul(ls[:1, :], onesb[:], pt[:, 1, :],
                                     start=False, stop=last)
                else:
                    nc.tensor.matmul(ys[:], vsb[:, k0 : k0 + 2, :], pt[:],
                                     start=first, stop=last, perf_mode=DR)
                    nc.tensor.matmul(ls[:], ones8[:], pt[:],
                                     start=first, stop=last, perf_mode=DR)
                if last:
                    yo = outp.tile([128, 512], F32, tag="yo")
                    lo = outp.tile([1, 512], F32, tag="lo")
                    if qb == jobs[-1][0]:
                        # final drain: halve the yo copy so the first acc DMA
                        # overlaps the second half, and put the lo copy on the
                        # now-idle ACT engine so the l DMA runs in parallel
                        nc.vector.tensor_copy(yo[:, :256], ys[:, :256])
                        nc.sync.dma_start(
                            acc_out[:, 512 * qb : 512 * qb + 256], yo[:, :256])
                        nc.scalar.copy(lo[:], ls[:1, :])
                        nc.gpsimd.dma_start(
                            l_out[:, 512 * qb : 512 * (qb + 1)], lo[:])
                        nc.vector.tensor_copy(yo[:, 256:], ys[:, 256:])
                        nc.sync.dma_start(
                            acc_out[:, 512 * qb + 256 : 512 * (qb + 1)],
                            yo[:, 256:])
                    else:
                        nc.vector.tensor_copy(yo[:], ys[:])
                        nc.sync.dma_start(
                            acc_out[:, 512 * qb : 512 * (qb + 1)], yo[:])
                        nc.vector.tensor_copy(lo[:], ls[:1, :])
                        nc.gpsimd.dma_start(
                            l_out[:, 512 * qb : 512 * (qb + 1)], lo[:])

            s_stage(0)
            for j in range(n):
                for u in units.get(j, []):
                    u()
                if j + 1 < n:
                    s_stage(j + 1)
                av_stage(j)

    nc.compile()
    return nc


def _prep_x8(xpart):
    """[Tpart, E] f32 -> fp8 tiled [128, tb, ec, 512] host layout."""
    tb = xpart.shape[0] // 512
    a = xpart.T.astype(NP8)                         # [E, Tpart]
    a = a.reshape(EC, 128, tb, 512).transpose(1, 2, 0, 3)
    return np.ascontiguousarray(a)


def _prep_xb0w(xpart):
    """[512, E] f32 -> bf16 [128, ec, 512]."""
    a = xpart.T.astype(ml_dtypes.bfloat16)          # [E, 512]
    a = a.reshape(EC, 128, 512).transpose(1, 0, 2)
    return np.ascontiguousarray(a)


def _prep_w(w, dt):
    """[H, E] f32 -> [128, ec, H] (w.T chunked)."""
    a = w.T.astype(dt)                              # [E, H]
    a = a.reshape(EC, 128, H).transpose(1, 0, 2)
    return np.ascontiguousarray(a)


def kernel(x_in, Wq, Wk, Wv):
    B, T_, E_ = x_in.shape
    assert (B, T_, E_) == (4, T, E)
    nc = _CACHED.get("nc")
    if nc is None:
        nc = _CACHED["nc"] = _build()

    w8 = {n: _prep_w(w, NP8) for n, w in (("q", Wq), ("k", Wk), ("v", Wv))}
    wb = {n: _prep_w(w, ml_dtypes.bfloat16)
          for n, w in (("q", Wq), ("k", Wk), ("v", Wv))}
    in_maps = []
    for c in range(8):
        b, h = c // 2, c % 2
        xb = np.asarray(x_in[b], dtype=np.float32)
        c0, c1 = xb[:CH], xb[CH:]
        own = c0 if h == 0 else c1
        xq = np.concatenate([own, c1], axis=0)        # [4096, E]
        rk = xb[0:RK] if h == 0 else xb[RK : 2 * RK]  # [1024, E]
        m = {"xq_in": _prep_x8(xq), "xrk_in": _prep_x8(rk),
             "xb0w_in": _prep_xb0w(own[:512])}
        for n in ("q", "k", "v"):
            m[f"w8{n}_in"] = w8[n]
            m[f"wb{n}_in"] = wb[n]
        in_maps.append(m)

    kw = {}
    if TRACE:
        kw = {"trace": True, "trace_cores": TRACE_CORES}
    res = run_bass_kernel_spmd(nc, in_maps, core_ids=list(range(8)), **kw)
    global LAST_RESULTS
    LAST_RESULTS = res

    y = np.empty((B, T, H), dtype=np.float32)
    for b in range(4):
        r0, r1 = res.results[2 * b], res.results[2 * b + 1]
        a0, l0 = r0["acc_out"], r0["l_out"][0]
        a1, l1 = r1["acc_out"], r1["l_out"][0]
        y[b, :CH] = (a0[:, :CH] / l0[:CH]).T
        acc = a0[:, CH:] + a1[:, :CH] + a1[:, CH:]
        l = l0[CH:] + l1[:CH] + l1[CH:]
        y[b, CH:] = (acc / l).T
    return y
